# revision 1
# baseline (speedup 1.0000x reference)
"""MoE routing kernel for Trainium2, expert-parallel across 8 NeuronCores.

Sharding: core c owns experts [8c, 8c+8). The gate/top-k/dispatch-position
computation runs on host as part of the sharding step; each core receives its
experts' dispatched token rows (transposed, bf16), its expert weights, and a
slice of tokens for the (replicated-weight) shared expert. Device computes the
grouped SwiGLU expert GEMMs + shared expert. Host gathers per-slot outputs and
does the weighted combine (unshard).
"""

import os

import numpy as np
import ml_dtypes

import bass_rust
import concourse.bass as bass
import concourse.mybir as mybir
from concourse.tile import TileContext
from concourse.vector_clock import ScopedClock
from concourse.bass_utils import run_bass_kernel_spmd

B, T, C = 2, 2048, 2048
N = B * T
E, H, HS = 64, 256, 512
TOPK = 6
NCORES = 8
ELOC = E // NCORES  # 8 experts per core
NLOC = N // NCORES  # 512 tokens per core for the shared expert
BF16 = mybir.dt.bfloat16
F32 = mybir.dt.float32
P = 128

_BF16_NP = ml_dtypes.bfloat16


# --------------------------------------------------------------------------
# Tile tail-drain fix: this walrus build allows at most one semaphore wait per
# instruction (none on Drain). Tile's end-of-context drain carries the whole
# global clock; emit a chain of single-wait NOPs on SP instead.
# --------------------------------------------------------------------------
def _patched_drain_and_barrier(self, tick_clock, wait_clock):
    carrier = self.nc.sync.nop(nofuse=True, hint="tail_wait_0")
    wait_clock.add_sem_waits(carrier.ins, ScopedClock({None: tick_clock.global_clock}))
    si = carrier.ins.sync_info
    waits = list(si.on_wait) if si else []
    upds = list(si.on_update) if si else []
    carrier.ins.sync_info = bass_rust.SyncInfo(on_wait=waits[:1], on_update=upds)
    for i, w in enumerate(waits[1:]):
        n2 = self.nc.sync.nop(nofuse=True, hint=f"tail_wait_{i + 1}")
        n2.ins.sync_info = bass_rust.SyncInfo(on_wait=[w], on_update=[])

    self.nc.sync.drain()
    self.nc.all_engine_barrier()
    assert self.sems is not None
    popped = self.nc._tile_sem_poison_stack.pop()
    assert popped is self._sem_poison
    self.nc.clear_and_free_semaphores(list(self.sems.allocated().values()))
    self.nc.all_engine_barrier()


_orig_add_instruction = TileContext._add_instruction


def _patched_add_instruction(self, inst):
    si = getattr(inst, "sync_info", None)
    if si is not None and len(si.on_wait) > 1:
        waits = list(si.on_wait)
        for w in waits[:-1]:
            nop = mybir.InstNoOp(
                name=self.nc.get_next_instruction_name(), ins=[], outs=[])
            nop.engine = inst.engine
            nop.sync_info = bass_rust.SyncInfo(on_wait=[w], on_update=[])
            _orig_add_instruction(self, nop)
        inst.sync_info = bass_rust.SyncInfo(
            on_wait=[waits[-1]], on_update=list(si.on_update))
    _orig_add_instruction(self, inst)


def _install_drain_fix():
    if getattr(TileContext, "_drain_fix_installed", False):
        return
    TileContext._drain_and_barrier = _patched_drain_and_barrier
    TileContext._add_instruction = _patched_add_instruction
    TileContext._drain_fix_installed = True


# --------------------------------------------------------------------------
# Device kernel
# --------------------------------------------------------------------------
_BUILD_CACHE = {}


def _build(cap):
    """Build the per-core Bass program; cap = padded per-expert capacity."""
    _install_drain_fix()
    nc = bass.Bass()

    xdT = nc.declare_dram_parameter("xdT", [ELOC, C, cap], BF16, isOutput=False)
    wup = nc.declare_dram_parameter("wup", [ELOC, C, 2 * H], BF16, isOutput=False)
    wdn = nc.declare_dram_parameter("wdn", [ELOC, H, C], BF16, isOutput=False)
    xsT = nc.declare_dram_parameter("xsT", [C, NLOC], BF16, isOutput=False)
    wsu = nc.declare_dram_parameter("wsu", [C, 2 * HS], BF16, isOutput=False)
    wsd = nc.declare_dram_parameter("wsd", [HS, C], BF16, isOutput=False)
    yr = nc.declare_dram_parameter("yr", [ELOC * cap, C], BF16, isOutput=True)
    ysh = nc.declare_dram_parameter("ysh", [NLOC, C], BF16, isOutput=True)

    KC = C // P          # 16 contraction chunks over C
    MU = (2 * H) // P    # 4 output chunks of up-proj (2H = 512)
    KH = H // P          # 2 contraction chunks over H
    NCC = C // 512       # 4 output column chunks of down-proj
    SC = cap // P        # slot chunks per expert
    assert cap % P == 0

    with TileContext(nc) as tc:
        with (
            tc.tile_pool(name="wu_sb", bufs=24) as wu_pool,
            tc.tile_pool(name="xd_sb", bufs=24) as xd_pool,
            tc.tile_pool(name="wd_sb", bufs=4) as wd_pool,
            tc.tile_pool(name="h_sb", bufs=8) as h_pool,
            tc.tile_pool(name="sg_sb", bufs=4) as sg_pool,
            tc.tile_pool(name="o_sb", bufs=6) as o_pool,
            tc.tile_pool(name="sh_sb", bufs=KC) as sh_pool,
            tc.tile_pool(name="pu", bufs=6, space="PSUM") as pu_pool,
            tc.tile_pool(name="pd", bufs=2, space="PSUM") as pd_pool,
        ):
            # ---------------- shared expert (512 local tokens) -------------
            xs_tiles = []
            for k in range(KC):
                t = sh_pool.tile([P, NLOC], BF16, tag="xs")
                nc.sync.dma_start(out=t[:], in_=xsT[k * P:(k + 1) * P, :])
                xs_tiles.append(t)

            wsu_tiles = []
            for k in range(KC):
                t = sh_pool.tile([P, 2 * HS], BF16, tag="wsu")
                nc.sync.dma_start(out=t[:], in_=wsu[k * P:(k + 1) * P, :])
                wsu_tiles.append(t)

            hsh_tiles = []  # [HS part chunks (4), NLOC] bf16, h = silu(g_s)*y_s
            for half in range(2):  # process 2H_S=1024 in halves of 512 cols
                ps_tiles = []
                for m in range(4):
                    mm = half * 4 + m
                    pt = pu_pool.tile([P, NLOC], F32, space="PSUM", tag="pu")
                    for k in range(KC):
                        nc.tensor.matmul(
                            out=pt[:],
                            lhsT=wsu_tiles[k][:, mm * P:(mm + 1) * P],
                            rhs=xs_tiles[k][:],
                            start=(k == 0), stop=(k == KC - 1))
                    ps_tiles.append(pt)
                if half == 0:
                    # channels 0:512 = y_s (chunk order: y first); move out of
                    # PSUM so the second half can reuse the banks
                    y_s_tiles = []
                    for j in range(4):
                        yt = sg_pool.tile([P, NLOC], F32, tag="ys")
                        nc.vector.tensor_copy(out=yt[:], in_=ps_tiles[j][:])
                        y_s_tiles.append(yt)
                else:
                    # channels 512:1024 = g_s; h = silu(g_s) * y_s
                    for j in range(4):
                        sg = sg_pool.tile([P, NLOC], F32, tag="sg")
                        nc.scalar.activation(sg[:], ps_tiles[j][:],
                                             mybir.ActivationFunctionType.Silu)
                        ht = h_pool.tile([P, NLOC], BF16, tag="h")
                        nc.vector.tensor_mul(ht[:], sg[:], y_s_tiles[j][:])
                        hsh_tiles.append(ht)

            wsd_tiles = []
            for k in range(4):  # HS = 512 -> 4 chunks
                t = wd_pool.tile([P, C], BF16, tag="wsd")
                nc.sync.dma_start(out=t[:], in_=wsd[k * P:(k + 1) * P, :])
                wsd_tiles.append(t)
            for mt in range(NLOC // P):  # 4 token chunks
                for ncc in range(NCC):
                    pt = pd_pool.tile([P, 512], F32, space="PSUM", tag="pd")
                    for k in range(4):
                        nc.tensor.matmul(
                            out=pt[:],
                            lhsT=hsh_tiles[k][:, mt * P:(mt + 1) * P],
                            rhs=wsd_tiles[k][:, ncc * 512:(ncc + 1) * 512],
                            start=(k == 0), stop=(k == 3))
                    ot = o_pool.tile([P, 512], BF16, tag="osh")
                    nc.vector.tensor_copy(out=ot[:], in_=pt[:])
                    nc.scalar.dma_start(
                        out=ysh[mt * P:(mt + 1) * P, ncc * 512:(ncc + 1) * 512],
                        in_=ot[:])

            # ---------------- routed experts ------------------------------
            for e in range(ELOC):
                # up-projection: psum[m] = [128 of 2H, cap slots]
                xd_tiles = []
                for k in range(KC):
                    t = xd_pool.tile([P, cap], BF16, tag="xd")
                    nc.sync.dma_start(
                        out=t[:], in_=xdT[e, k * P:(k + 1) * P, :])
                    xd_tiles.append(t)
                wu_tiles = []
                for k in range(KC):
                    wt = wu_pool.tile([P, 2 * H], BF16, tag="wu")
                    nc.sync.dma_start(out=wt[:], in_=wup[e, k * P:(k + 1) * P, :])
                    wu_tiles.append(wt)
                up_tiles = []
                for m in range(MU):
                    pt = pu_pool.tile([P, cap], F32, space="PSUM", tag="pu")
                    for k in range(KC):
                        nc.tensor.matmul(
                            out=pt[:],
                            lhsT=wu_tiles[k][:, m * P:(m + 1) * P],
                            rhs=xd_tiles[k][:],
                            start=(k == 0), stop=(k == KC - 1))
                    up_tiles.append(pt)
                # g = chunks 0..1 (first 256 channels), v = chunks 2..3
                h_tiles = []
                for j in range(KH):
                    sg = sg_pool.tile([P, cap], F32, tag="sg2")
                    nc.scalar.activation(sg[:], up_tiles[j][:],
                                         mybir.ActivationFunctionType.Silu)
                    ht = h_pool.tile([P, cap], BF16, tag="h2")
                    nc.vector.tensor_mul(ht[:], sg[:], up_tiles[KH + j][:])
                    h_tiles.append(ht)
                # down-projection: lhsT = h slot-chunk, rhs = w_down columns
                wd_tiles = []
                for k in range(KH):
                    t = wd_pool.tile([P, C], BF16, tag="wd")
                    nc.sync.dma_start(
                        out=t[:], in_=wdn[e, k * P:(k + 1) * P, :])
                    wd_tiles.append(t)
                for ms in range(SC):
                    for ncc in range(NCC):
                        pt = pd_pool.tile([P, 512], F32, space="PSUM", tag="pd")
                        for k in range(KH):
                            nc.tensor.matmul(
                                out=pt[:],
                                lhsT=h_tiles[k][:, ms * P:(ms + 1) * P],
                                rhs=wd_tiles[k][:, ncc * 512:(ncc + 1) * 512],
                                start=(k == 0), stop=(k == KH - 1))
                        ot = o_pool.tile([P, 512], BF16, tag="ord")
                        nc.vector.tensor_copy(out=ot[:], in_=pt[:])
                        row0 = e * cap + ms * P
                        nc.scalar.dma_start(
                            out=yr[row0:row0 + P, ncc * 512:(ncc + 1) * 512],
                            in_=ot[:])
    return nc


# --------------------------------------------------------------------------
# Host wrapper
# --------------------------------------------------------------------------
def kernel(x, w_gate, w_shared_up, w_shared_down, w_up, w_down):
    x_flat = x.reshape(-1, C)

    # ---- gate: sigmoid scores, top-6, normalized weights (f64 for a stable
    # ordering; ties in the fp32 reference are measure-zero) ----
    logits = x_flat.astype(np.float64) @ w_gate.astype(np.float64)
    scores = 1.0 / (1.0 + np.exp(-logits))
    topk_idx = np.argsort(-scores, axis=-1, kind="stable")[:, :TOPK]
    w = np.take_along_axis(scores, topk_idx, axis=-1)
    w = w / w.sum(-1, keepdims=True)

    # ---- dispatch positions (stable within each expert, slot-major order) --
    flat_e = topk_idx.reshape(-1)
    order = np.argsort(flat_e, kind="stable")
    sorted_e = flat_e[order]
    group_start = np.searchsorted(sorted_e, np.arange(E))
    pos = np.empty(N * TOPK, dtype=np.int64)
    pos[order] = np.arange(N * TOPK) - group_start[sorted_e]
    counts = np.bincount(flat_e, minlength=E)

    cap = 512
    mx = int(counts.max())
    if mx > cap:
        cap = ((mx + P - 1) // P) * P

    # ---- build per-core inputs ----
    xT_bf = np.ascontiguousarray(x_flat.T).astype(_BF16_NP)  # [C, N]
    wup_bf = w_up.astype(_BF16_NP)
    wdn_bf = w_down.astype(_BF16_NP)
    wsu_bf = w_shared_up.astype(_BF16_NP)
    wsd_bf = w_shared_down.astype(_BF16_NP)

    token_of_slot = np.arange(N * TOPK) // TOPK
    in_maps = []
    expert_tokens = []
    for e in range(E):
        slots = order[group_start[e]: group_start[e] + counts[e]]
        expert_tokens.append(token_of_slot[slots])
    for c in range(NCORES):
        xdT = np.zeros((ELOC, C, cap), dtype=_BF16_NP)
        for j in range(ELOC):
            tok = expert_tokens[c * ELOC + j]
            xdT[j][:, : len(tok)] = xT_bf[:, tok]
        xsT = np.ascontiguousarray(xT_bf[:, c * NLOC:(c + 1) * NLOC])
        in_maps.append({
            "xdT": xdT,
            "wup": wup_bf[c * ELOC:(c + 1) * ELOC],
            "wdn": wdn_bf[c * ELOC:(c + 1) * ELOC],
            "xsT": xsT,
            "wsu": wsu_bf,
            "wsd": wsd_bf,
        })

    if cap not in _BUILD_CACHE:
        _BUILD_CACHE[cap] = _build(cap)
    nc = _BUILD_CACHE[cap]

    res = run_bass_kernel_spmd(nc, in_maps, list(range(NCORES)))
    if res.exec_time_ns is not None:
        print(f"HW exec time: {res.exec_time_ns} ns", flush=True)

    # ---- host combine (unshard): gather per-slot rows, weight, sum ----
    yr_all = np.concatenate(
        [r["yr"].reshape(ELOC, cap, C) for r in res.results], axis=0)
    y_ts = yr_all[flat_e, pos].astype(np.float32)          # [N*K, C]
    routed = (y_ts.reshape(N, TOPK, C)
              * w.reshape(N, TOPK, 1).astype(np.float32)).sum(axis=1)
    shared = np.concatenate([r["ysh"] for r in res.results], axis=0).astype(np.float32)
    return (shared + routed).reshape(B, T, C).astype(np.float32)



# revision 4
# speedup vs baseline: 1.3119x; 1.3119x over previous
"""MoE routing kernel for Trainium2, expert-parallel across 8 NeuronCores.

Sharding: experts are sorted by dispatch count and dealt round-robin so that
slot position j on every core has the same padded capacity caps[j] (baked into
the SPMD program). The gate/top-k/dispatch runs on host as part of sharding;
each core receives its experts' dispatched token rows in a partition-major
layout (one large contiguous per-partition block per tensor, so every
dma_start moves 4-16KB per partition), its expert weights, and a slice of
tokens for the (replicated-weight) shared expert. Device computes the grouped
SwiGLU expert GEMMs + shared expert with slots always on the moving dim (no
partial-tile matmul waste). Host gathers per-slot outputs and does the
weighted combine (unshard).
"""

import numpy as np
import ml_dtypes

import bass_rust
import concourse.bass as bass
import concourse.mybir as mybir
from concourse.tile import TileContext
from concourse.vector_clock import ScopedClock
from concourse.bass_utils import run_bass_kernel_spmd

B, T, C = 2, 2048, 2048
N = B * T
E, H, HS = 64, 256, 512
TOPK = 6
NCORES = 8
ELOC = E // NCORES  # 8 experts per core
NLOC = N // NCORES  # 512 tokens per core for the shared expert
BF16 = mybir.dt.bfloat16
F32 = mybir.dt.float32
P = 128
KC = C // P  # 16 contraction chunks over C

_BF16_NP = ml_dtypes.bfloat16

# shared-up pass layout: pass 0 computes output chunks m in {0,1,4,5} of the
# 2*HS=1024 columns (y halves 0,1 + gate halves 0,1), pass 1 m in {2,3,6,7}
_MSEL = [[0, 1, 4, 5], [2, 3, 6, 7]]


# --------------------------------------------------------------------------
# Tile tail-drain fix: this walrus build allows at most one semaphore wait per
# instruction (none on Drain). Tile's end-of-context drain carries the whole
# global clock; emit a chain of single-wait NOPs on SP instead.
# --------------------------------------------------------------------------
def _patched_drain_and_barrier(self, tick_clock, wait_clock):
    carrier = self.nc.sync.nop(nofuse=True, hint="tail_wait_0")
    wait_clock.add_sem_waits(carrier.ins, ScopedClock({None: tick_clock.global_clock}))
    si = carrier.ins.sync_info
    waits = list(si.on_wait) if si else []
    upds = list(si.on_update) if si else []
    carrier.ins.sync_info = bass_rust.SyncInfo(on_wait=waits[:1], on_update=upds)
    for i, w in enumerate(waits[1:]):
        n2 = self.nc.sync.nop(nofuse=True, hint=f"tail_wait_{i + 1}")
        n2.ins.sync_info = bass_rust.SyncInfo(on_wait=[w], on_update=[])

    self.nc.sync.drain()
    self.nc.all_engine_barrier()
    assert self.sems is not None
    popped = self.nc._tile_sem_poison_stack.pop()
    assert popped is self._sem_poison
    self.nc.clear_and_free_semaphores(list(self.sems.allocated().values()))
    self.nc.all_engine_barrier()


_orig_add_instruction = TileContext._add_instruction


def _patched_add_instruction(self, inst):
    si = getattr(inst, "sync_info", None)
    if si is not None and len(si.on_wait) > 1:
        waits = list(si.on_wait)
        for w in waits[:-1]:
            nop = mybir.InstNoOp(
                name=self.nc.get_next_instruction_name(), ins=[], outs=[])
            nop.engine = inst.engine
            nop.sync_info = bass_rust.SyncInfo(on_wait=[w], on_update=[])
            _orig_add_instruction(self, nop)
        inst.sync_info = bass_rust.SyncInfo(
            on_wait=[waits[-1]], on_update=list(si.on_update))
    _orig_add_instruction(self, inst)


def _install_drain_fix():
    if getattr(TileContext, "_drain_fix_installed", False):
        return
    TileContext._drain_and_barrier = _patched_drain_and_barrier
    TileContext._add_instruction = _patched_add_instruction
    TileContext._drain_fix_installed = True


# --------------------------------------------------------------------------
# Device kernel
# --------------------------------------------------------------------------
_BUILD_CACHE = {}


def _build(caps):
    """Per-core Bass program; caps[j] = padded capacity of slot position j."""
    _install_drain_fix()
    nc = bass.Bass()

    offs = [0]
    for cp in caps:
        offs.append(offs[-1] + cp)
    S = offs[-1]

    # all HBM tensors are partition-major: [128, X] with large contiguous
    # per-partition runs so DMA packets are 4-16KB
    xdh = nc.declare_dram_parameter("xdh", [P, KC * S], BF16, isOutput=False)
    wuh = nc.declare_dram_parameter("wuh", [P, ELOC * KC * 2 * H], BF16, isOutput=False)
    wdh = nc.declare_dram_parameter("wdh", [P, ELOC * 2 * C], BF16, isOutput=False)
    xsh = nc.declare_dram_parameter("xsh", [P, KC * NLOC], BF16, isOutput=False)
    wsuh = nc.declare_dram_parameter("wsuh", [P, KC * 2 * HS], BF16, isOutput=False)
    wsdh = nc.declare_dram_parameter("wsdh", [P, 4 * C], BF16, isOutput=False)
    yrh = nc.declare_dram_parameter("yrh", [P, KC * S], BF16, isOutput=True)
    ysh = nc.declare_dram_parameter("ysh", [NLOC, C], BF16, isOutput=True)

    with TileContext(nc) as tc:
        with (
            tc.tile_pool(name="xsg_sb", bufs=4) as xsg_pool,
            tc.tile_pool(name="wsug_sb", bufs=6) as wsug_pool,
            tc.tile_pool(name="wsd_sb", bufs=1) as wsd_pool,
            tc.tile_pool(name="hsh_sb", bufs=1) as hsh_pool,
            tc.tile_pool(name="osh_sb", bufs=2) as osh_pool,
            tc.tile_pool(name="xd_sb", bufs=2) as xd_pool,
            tc.tile_pool(name="wu_sb", bufs=2) as wu_pool,
            tc.tile_pool(name="wd_sb", bufs=2) as wd_pool,
            tc.tile_pool(name="yo_sb", bufs=2) as yo_pool,
            tc.tile_pool(name="h_sb", bufs=2) as h_pool,
            tc.tile_pool(name="sg_sb", bufs=2) as sg_pool,
            tc.tile_pool(name="pu", bufs=4, space="PSUM") as pu_pool,
            tc.tile_pool(name="pd", bufs=3, space="PSUM") as pd_pool,
        ):
            # ---------------- shared expert loads (k-grouped) --------------
            xs_g = []
            for g in range(4):
                t = xsg_pool.tile([P, 4 * NLOC], BF16, tag="xsg")
                nc.sync.dma_start(out=t[:], in_=xsh[:, g * 4 * NLOC:(g + 1) * 4 * NLOC])
                xs_g.append(t)
            wsu_pg = [[None] * 4 for _ in range(2)]
            for p in range(2):
                for g in range(4):
                    t = wsug_pool.tile([P, 4 * 512], BF16, tag="wsug")
                    base = p * 8192 + g * 2048
                    nc.sync.dma_start(out=t[:], in_=wsuh[:, base:base + 2048])
                    wsu_pg[p][g] = t
            wsd_t = wsd_pool.tile([P, 4 * C], BF16, tag="wsd")
            nc.sync.dma_start(out=wsd_t[:], in_=wsdh[:])

            # ---------------- shared up: 2 passes x 4 psums ----------------
            # hsh holds h = silu(g_s)*y_s as 4 chunks of [128, 512] columns
            hsh = hsh_pool.tile([P, 4 * NLOC], BF16, tag="hsh")
            for p in range(2):
                ps = [pu_pool.tile([P, NLOC], F32, space="PSUM", tag="pu",
                                   name=f"ps_{p}_{q}")
                      for q in range(4)]
                for g in range(4):
                    for kk in range(4):
                        k = 4 * g + kk
                        for q in range(4):
                            nc.tensor.matmul(
                                out=ps[q][:],
                                lhsT=wsu_pg[p][g][:, kk * 512 + q * P:
                                                  kk * 512 + (q + 1) * P],
                                rhs=xs_g[g][:, kk * NLOC:(kk + 1) * NLOC],
                                start=(k == 0), stop=(k == KC - 1))
                # pairs: ps[i] = y chunk (2p+i), ps[2+i] = gate chunk (2p+i)
                for i in range(2):
                    hc = 2 * p + i
                    sg = sg_pool.tile([P, NLOC], F32, tag="sg")
                    nc.scalar.activation(sg[:], ps[2 + i][:],
                                         mybir.ActivationFunctionType.Silu)
                    nc.vector.tensor_mul(
                        hsh[:, hc * NLOC:(hc + 1) * NLOC], sg[:], ps[i][:])

            # ---------------- shared down: tokens as out rows --------------
            for mt in range(4):
                osh = osh_pool.tile([P, C], BF16, tag="osh")
                for ncc in range(4):
                    pd = pd_pool.tile([P, 512], F32, space="PSUM", tag="pd")
                    for kh in range(4):
                        nc.tensor.matmul(
                            out=pd[:],
                            lhsT=hsh[:, kh * NLOC + mt * P:kh * NLOC + (mt + 1) * P],
                            rhs=wsd_t[:, kh * C + ncc * 512:kh * C + (ncc + 1) * 512],
                            start=(kh == 0), stop=(kh == 3))
                    nc.vector.tensor_copy(out=osh[:, ncc * 512:(ncc + 1) * 512],
                                          in_=pd[:])
                nc.scalar.dma_start(out=ysh[mt * P:(mt + 1) * P, :], in_=osh[:])

            # ---------------- routed experts ------------------------------
            for j in range(ELOC):
                cap = caps[j]
                base = KC * offs[j]
                xd = xd_pool.tile([P, KC * cap], BF16, tag="xd")
                nc.sync.dma_start(out=xd[:], in_=xdh[:, base:base + KC * cap])
                wu = wu_pool.tile([P, KC * 2 * H], BF16, tag="wu")
                nc.sync.dma_start(
                    out=wu[:], in_=wuh[:, j * KC * 2 * H:(j + 1) * KC * 2 * H])
                wd = wd_pool.tile([P, 2 * C], BF16, tag="wd")
                nc.sync.dma_start(out=wd[:], in_=wdh[:, j * 2 * C:(j + 1) * 2 * C])
                yo = yo_pool.tile([P, KC * cap], BF16, tag="yo")

                for sb in range(0, cap, 512):
                    w_ = min(512, cap - sb)
                    # up: psum[m] = [128 of 2H, w_ slots]; m 0..1 = gate, 2..3 = v
                    pus = [pu_pool.tile([P, w_], F32, space="PSUM", tag="pu",
                                        name=f"pus_{j}_{sb}_{m}")
                           for m in range(4)]
                    for k in range(KC):
                        for m in range(4):
                            nc.tensor.matmul(
                                out=pus[m][:],
                                lhsT=wu[:, k * 512 + m * P:k * 512 + (m + 1) * P],
                                rhs=xd[:, k * cap + sb:k * cap + sb + w_],
                                start=(k == 0), stop=(k == KC - 1))
                    h = h_pool.tile([P, 2 * w_], BF16, tag="h")
                    for i in range(2):
                        sg = sg_pool.tile([P, w_], F32, tag="sg")
                        nc.scalar.activation(sg[:], pus[i][:],
                                             mybir.ActivationFunctionType.Silu)
                        nc.vector.tensor_mul(
                            h[:, i * w_:(i + 1) * w_], sg[:], pus[2 + i][:])
                    # down: out = [128 of C, w_ slots] per C chunk cc
                    for cc in range(KC):
                        pd = pd_pool.tile([P, w_], F32, space="PSUM", tag="pd")
                        for kh in range(2):
                            nc.tensor.matmul(
                                out=pd[:],
                                lhsT=wd[:, kh * C + cc * P:kh * C + (cc + 1) * P],
                                rhs=h[:, kh * w_:(kh + 1) * w_],
                                start=(kh == 0), stop=(kh == 1))
                        nc.vector.tensor_copy(
                            out=yo[:, cc * cap + sb:cc * cap + sb + w_], in_=pd[:])
                nc.scalar.dma_start(out=yrh[:, base:base + KC * cap], in_=yo[:])
    return nc


# --------------------------------------------------------------------------
# Host wrapper
# --------------------------------------------------------------------------
def _pm(a, nchunk):
    """[nchunk*128, X] row-major -> partition-major [128, nchunk*X]."""
    x = a.shape[1]
    return np.ascontiguousarray(
        a.reshape(nchunk, P, x).transpose(1, 0, 2)).reshape(P, nchunk * x)


def kernel(x, w_gate, w_shared_up, w_shared_down, w_up, w_down):
    x_flat = x.reshape(-1, C)

    # ---- gate: sigmoid scores, top-6, normalized weights (f64 for a stable
    # ordering; ties in the fp32 reference are measure-zero) ----
    logits = x_flat.astype(np.float64) @ w_gate.astype(np.float64)
    scores = 1.0 / (1.0 + np.exp(-logits))
    topk_idx = np.argsort(-scores, axis=-1, kind="stable")[:, :TOPK]
    w = np.take_along_axis(scores, topk_idx, axis=-1)
    w = w / w.sum(-1, keepdims=True)

    # ---- dispatch positions (stable within each expert, slot-major order) --
    flat_e = topk_idx.reshape(-1)
    order = np.argsort(flat_e, kind="stable")
    sorted_e = flat_e[order]
    group_start = np.searchsorted(sorted_e, np.arange(E))
    counts = np.bincount(flat_e, minlength=E)

    token_of_slot = np.arange(N * TOPK) // TOPK
    expert_slots = []   # flat (token,k) slot ids, dispatch order, per expert
    expert_tokens = []
    for e in range(E):
        slots = order[group_start[e]: group_start[e] + counts[e]]
        expert_slots.append(slots)
        expert_tokens.append(token_of_slot[slots])

    # ---- balanced expert->core assignment: sort by count desc, deal 8 at a
    # time; slot position j has the same padded cap on every core ----
    ranks = np.argsort(-counts, kind="stable")
    expert_of = [[int(ranks[8 * j + c]) for j in range(ELOC)]
                 for c in range(NCORES)]
    caps = tuple(
        max(8, int(-(-int(counts[ranks[8 * j]]) // 8) * 8)) for j in range(ELOC))
    offs = [0]
    for cp in caps:
        offs.append(offs[-1] + cp)
    S = offs[-1]

    # ---- build per-core inputs (partition-major bf16) ----
    xT_bf = np.ascontiguousarray(x_flat.T).astype(_BF16_NP)  # [C, N]
    wsu_f = w_shared_up.astype(_BF16_NP)
    wsd_f = w_shared_down.astype(_BF16_NP)

    # shared-up weights in pass/k/q-major order (see _MSEL)
    msel = np.array(_MSEL).reshape(-1)
    wr = wsu_f.reshape(KC, P, 8, P)[:, :, msel, :]          # [k, p, 8, 128]
    wsuh = np.ascontiguousarray(
        wr.reshape(KC, P, 2, 4 * P).transpose(1, 2, 0, 3)).reshape(P, KC * 2 * HS)
    wsdh = _pm(wsd_f, 4)

    in_maps = []
    for c in range(NCORES):
        xd_blocks = []
        wu_blocks = []
        wd_blocks = []
        for j in range(ELOC):
            e = expert_of[c][j]
            tok = expert_tokens[e]
            n = len(tok)
            blk = np.zeros((P, KC, caps[j]), dtype=_BF16_NP)
            blk[:, :, :n] = xT_bf[:, tok].reshape(KC, P, n).transpose(1, 0, 2)
            xd_blocks.append(blk.reshape(P, -1))
            wu_blocks.append(_pm(w_up[e].astype(_BF16_NP), KC))
            wd_blocks.append(_pm(w_down[e].astype(_BF16_NP), 2))
        xsh = _pm(np.ascontiguousarray(
            xT_bf[:, c * NLOC:(c + 1) * NLOC]), KC)
        in_maps.append({
            "xdh": np.concatenate(xd_blocks, axis=1),
            "wuh": np.concatenate(wu_blocks, axis=1),
            "wdh": np.concatenate(wd_blocks, axis=1),
            "xsh": xsh,
            "wsuh": wsuh,
            "wsdh": wsdh,
        })

    if caps not in _BUILD_CACHE:
        _BUILD_CACHE[caps] = _build(caps)
    nc = _BUILD_CACHE[caps]

    res = run_bass_kernel_spmd(nc, in_maps, list(range(NCORES)))
    if res.exec_time_ns is not None:
        print(f"HW exec time: {res.exec_time_ns} ns", flush=True)

    # ---- host combine (unshard): gather per-slot rows, weight, sum ----
    y_ts = np.empty((N * TOPK, C), dtype=np.float32)
    for c in range(NCORES):
        yr = res.results[c]["yrh"]
        for j in range(ELOC):
            e = expert_of[c][j]
            n = int(counts[e])
            seg = yr[:, KC * offs[j]:KC * offs[j] + KC * caps[j]]
            seg = seg.reshape(P, KC, caps[j])[:, :, :n]
            y_ts[expert_slots[e]] = (
                seg.transpose(2, 1, 0).reshape(n, C).astype(np.float32))
    routed = (y_ts.reshape(N, TOPK, C)
              * w.reshape(N, TOPK, 1).astype(np.float32)).sum(axis=1)
    shared = np.concatenate(
        [r["ysh"] for r in res.results], axis=0).astype(np.float32)
    return (shared + routed).reshape(B, T, C).astype(np.float32)


# revision 5
# speedup vs baseline: 1.4269x; 1.0877x over previous
"""MoE routing kernel for Trainium2, expert-parallel across 8 NeuronCores.

Sharding: experts are sorted by dispatch count and dealt round-robin so that
slot position j on every core has the same padded capacity caps[j] (baked into
the SPMD program). The gate/top-k/dispatch runs on host as part of sharding;
each core receives its experts' dispatched token rows in a partition-major
layout (one large contiguous per-partition block per tensor, so every
dma_start moves 4-16KB per partition), its expert weights, and a slice of
tokens for the (replicated-weight) shared expert. Device computes the grouped
SwiGLU expert GEMMs + shared expert with slots always on the moving dim (no
partial-tile matmul waste) and SwiGLU pairs processed two-at-a-time so the
silu/mul chain overlaps the next pair's matmuls. Host gathers per-slot
outputs and does the weighted combine (unshard).
"""

import numpy as np
import ml_dtypes

import bass_rust
import concourse.bass as bass
import concourse.mybir as mybir
from concourse.tile import TileContext
from concourse.vector_clock import ScopedClock
from concourse.bass_utils import run_bass_kernel_spmd

B, T, C = 2, 2048, 2048
N = B * T
E, H, HS = 64, 256, 512
TOPK = 6
NCORES = 8
ELOC = E // NCORES  # 8 experts per core
NLOC = N // NCORES  # 512 tokens per core for the shared expert
BF16 = mybir.dt.bfloat16
F32 = mybir.dt.float32
P = 128
KC = C // P  # 16 contraction chunks over C

_BF16_NP = ml_dtypes.bfloat16

# shared-up pair order: pair i computes y chunk m=i and gate chunk m=4+i of
# the 2*HS=1024 up-projection columns
_MPAIR = [0, 4, 1, 5, 2, 6, 3, 7]


# --------------------------------------------------------------------------
# Tile tail-drain fix: this walrus build allows at most one semaphore wait per
# instruction (none on Drain). Tile's end-of-context drain carries the whole
# global clock; emit a chain of single-wait NOPs on SP instead.
# --------------------------------------------------------------------------
def _patched_drain_and_barrier(self, tick_clock, wait_clock):
    carrier = self.nc.sync.nop(nofuse=True, hint="tail_wait_0")
    wait_clock.add_sem_waits(carrier.ins, ScopedClock({None: tick_clock.global_clock}))
    si = carrier.ins.sync_info
    waits = list(si.on_wait) if si else []
    upds = list(si.on_update) if si else []
    carrier.ins.sync_info = bass_rust.SyncInfo(on_wait=waits[:1], on_update=upds)
    for i, w in enumerate(waits[1:]):
        n2 = self.nc.sync.nop(nofuse=True, hint=f"tail_wait_{i + 1}")
        n2.ins.sync_info = bass_rust.SyncInfo(on_wait=[w], on_update=[])

    self.nc.sync.drain()
    self.nc.all_engine_barrier()
    assert self.sems is not None
    popped = self.nc._tile_sem_poison_stack.pop()
    assert popped is self._sem_poison
    self.nc.clear_and_free_semaphores(list(self.sems.allocated().values()))
    self.nc.all_engine_barrier()


_orig_add_instruction = TileContext._add_instruction


def _patched_add_instruction(self, inst):
    si = getattr(inst, "sync_info", None)
    if si is not None and len(si.on_wait) > 1:
        waits = list(si.on_wait)
        for w in waits[:-1]:
            nop = mybir.InstNoOp(
                name=self.nc.get_next_instruction_name(), ins=[], outs=[])
            nop.engine = inst.engine
            nop.sync_info = bass_rust.SyncInfo(on_wait=[w], on_update=[])
            _orig_add_instruction(self, nop)
        inst.sync_info = bass_rust.SyncInfo(
            on_wait=[waits[-1]], on_update=list(si.on_update))
    _orig_add_instruction(self, inst)


def _install_drain_fix():
    if getattr(TileContext, "_drain_fix_installed", False):
        return
    TileContext._drain_and_barrier = _patched_drain_and_barrier
    TileContext._add_instruction = _patched_add_instruction
    TileContext._drain_fix_installed = True


# --------------------------------------------------------------------------
# Device kernel
# --------------------------------------------------------------------------
_BUILD_CACHE = {}


def _build(caps):
    """Per-core Bass program; caps[j] = padded capacity of slot position j."""
    _install_drain_fix()
    nc = bass.Bass()

    offs = [0]
    for cp in caps:
        offs.append(offs[-1] + cp)
    S = offs[-1]

    # all HBM tensors are partition-major: [128, X] with large contiguous
    # per-partition runs so DMA packets are 4-16KB
    xdh = nc.declare_dram_parameter("xdh", [P, KC * S], BF16, isOutput=False)
    wuh = nc.declare_dram_parameter("wuh", [P, ELOC * KC * 2 * H], BF16, isOutput=False)
    wdh = nc.declare_dram_parameter("wdh", [P, ELOC * 2 * C], BF16, isOutput=False)
    xsh = nc.declare_dram_parameter("xsh", [P, KC * NLOC], BF16, isOutput=False)
    wsuh = nc.declare_dram_parameter("wsuh", [P, KC * 2 * HS], BF16, isOutput=False)
    wsdh = nc.declare_dram_parameter("wsdh", [P, 4 * C], BF16, isOutput=False)
    yrh = nc.declare_dram_parameter("yrh", [P, KC * S], BF16, isOutput=True)
    ysh = nc.declare_dram_parameter("ysh", [NLOC, C], BF16, isOutput=True)

    with TileContext(nc) as tc:
        with (
            tc.tile_pool(name="xsg_sb", bufs=4) as xsg_pool,
            tc.tile_pool(name="wsug_sb", bufs=6) as wsug_pool,
            tc.tile_pool(name="wsd_sb", bufs=1) as wsd_pool,
            tc.tile_pool(name="hsh_sb", bufs=1) as hsh_pool,
            tc.tile_pool(name="osh_sb", bufs=2) as osh_pool,
            tc.tile_pool(name="xd_sb", bufs=3) as xd_pool,
            tc.tile_pool(name="wu_sb", bufs=3) as wu_pool,
            tc.tile_pool(name="wd_sb", bufs=2) as wd_pool,
            tc.tile_pool(name="yo_sb", bufs=2) as yo_pool,
            tc.tile_pool(name="h_sb", bufs=2) as h_pool,
            tc.tile_pool(name="sg_sb", bufs=2) as sg_pool,
            tc.tile_pool(name="pu", bufs=4, space="PSUM") as pu_pool,
            tc.tile_pool(name="pd", bufs=4, space="PSUM") as pd_pool,
        ):
            # ------------- shared expert loads (interleaved, k-grouped) -----
            # pair-0 weight group g then xs group g first, so the first
            # matmul's inputs land as early as possible
            xs_g = [None] * 4
            wsu_pg = [[None] * 4 for _ in range(4)]
            for g in range(4):
                t = wsug_pool.tile([P, 4 * 256], BF16, tag="wsug",
                                   name=f"wsu_0_{g}")
                nc.sync.dma_start(out=t[:], in_=wsuh[:, g * 1024:(g + 1) * 1024])
                wsu_pg[0][g] = t
                t2 = xsg_pool.tile([P, 4 * NLOC], BF16, tag="xsg",
                                   name=f"xs_{g}")
                nc.sync.dma_start(
                    out=t2[:], in_=xsh[:, g * 4 * NLOC:(g + 1) * 4 * NLOC])
                xs_g[g] = t2
            for pr in range(1, 4):
                for g in range(4):
                    t = wsug_pool.tile([P, 4 * 256], BF16, tag="wsug",
                                       name=f"wsu_{pr}_{g}")
                    base = pr * 4096 + g * 1024
                    nc.sync.dma_start(out=t[:], in_=wsuh[:, base:base + 1024])
                    wsu_pg[pr][g] = t
            wsd_t = wsd_pool.tile([P, 4 * C], BF16, tag="wsd")
            nc.sync.dma_start(out=wsd_t[:], in_=wsdh[:])

            # ------------- shared up: 4 pair passes x 2 psums ---------------
            # hsh holds h = silu(g_s)*y_s as 4 chunks of [128, 512] columns
            hsh = hsh_pool.tile([P, 4 * NLOC], BF16, tag="hsh")
            for pr in range(4):
                ps_y = pu_pool.tile([P, NLOC], F32, space="PSUM", tag="pu",
                                    name=f"ps_y{pr}")
                ps_g = pu_pool.tile([P, NLOC], F32, space="PSUM", tag="pu",
                                    name=f"ps_g{pr}")
                for g in range(4):
                    for kk in range(4):
                        k = 4 * g + kk
                        nc.tensor.matmul(
                            out=ps_y[:],
                            lhsT=wsu_pg[pr][g][:, kk * 256:kk * 256 + P],
                            rhs=xs_g[g][:, kk * NLOC:(kk + 1) * NLOC],
                            start=(k == 0), stop=(k == KC - 1))
                        nc.tensor.matmul(
                            out=ps_g[:],
                            lhsT=wsu_pg[pr][g][:, kk * 256 + P:(kk + 1) * 256],
                            rhs=xs_g[g][:, kk * NLOC:(kk + 1) * NLOC],
                            start=(k == 0), stop=(k == KC - 1))
                sg = sg_pool.tile([P, NLOC], F32, tag="sg", name=f"sgs_{pr}")
                nc.scalar.activation(sg[:], ps_g[:],
                                     mybir.ActivationFunctionType.Silu)
                nc.vector.tensor_mul(
                    hsh[:, pr * NLOC:(pr + 1) * NLOC], sg[:], ps_y[:])

            # ------------- shared down: tokens as out rows ------------------
            for mt in range(4):
                osh = osh_pool.tile([P, C], BF16, tag="osh", name=f"osh_{mt}")
                for ncc in range(4):
                    pd = pd_pool.tile([P, 512], F32, space="PSUM", tag="pd",
                                      name=f"pds_{mt}_{ncc}")
                    for kh in range(4):
                        nc.tensor.matmul(
                            out=pd[:],
                            lhsT=hsh[:, kh * NLOC + mt * P:kh * NLOC + (mt + 1) * P],
                            rhs=wsd_t[:, kh * C + ncc * 512:kh * C + (ncc + 1) * 512],
                            start=(kh == 0), stop=(kh == 3))
                    dst = osh[:, ncc * 512:(ncc + 1) * 512]
                    if ncc % 2 == 0:
                        nc.vector.tensor_copy(out=dst, in_=pd[:])
                    else:
                        nc.scalar.copy(dst, pd[:])
                nc.scalar.dma_start(out=ysh[mt * P:(mt + 1) * P, :], in_=osh[:])

            # ------------- routed experts ----------------------------------
            for j in range(ELOC):
                cap = caps[j]
                base = KC * offs[j]
                xd_h = []
                wu_h = []
                for hh in range(2):
                    t = xd_pool.tile([P, 8 * cap], BF16, tag="xd",
                                     name=f"xd_{j}_{hh}")
                    nc.sync.dma_start(
                        out=t[:],
                        in_=xdh[:, base + hh * 8 * cap:base + (hh + 1) * 8 * cap])
                    xd_h.append(t)
                    t2 = wu_pool.tile([P, 8 * 512], BF16, tag="wu",
                                      name=f"wu_{j}_{hh}")
                    wb = j * KC * 512 + hh * 8 * 512
                    nc.sync.dma_start(out=t2[:], in_=wuh[:, wb:wb + 8 * 512])
                    wu_h.append(t2)
                wd = wd_pool.tile([P, 2 * C], BF16, tag="wd", name=f"wd_{j}")
                nc.sync.dma_start(out=wd[:], in_=wdh[:, j * 2 * C:(j + 1) * 2 * C])
                yo = yo_pool.tile([P, KC * cap], BF16, tag="yo", name=f"yo_{j}")

                for sb in range(0, cap, 512):
                    w_ = min(512, cap - sb)
                    # up in two m-half passes; half h computes gate chunk m=h
                    # and v chunk m=2+h -> h chunk h; silu/mul of half 0
                    # overlaps half 1's matmuls
                    h = h_pool.tile([P, 2 * w_], BF16, tag="h",
                                    name=f"h_{j}_{sb}")
                    for hf in range(2):
                        p_g = pu_pool.tile([P, w_], F32, space="PSUM", tag="pu",
                                           name=f"pug_{j}_{sb}_{hf}")
                        p_v = pu_pool.tile([P, w_], F32, space="PSUM", tag="pu",
                                           name=f"puv_{j}_{sb}_{hf}")
                        for k in range(KC):
                            lb = (k % 8) * 512
                            rhs = xd_h[k // 8][:, (k % 8) * cap + sb:
                                               (k % 8) * cap + sb + w_]
                            nc.tensor.matmul(
                                out=p_g[:],
                                lhsT=wu_h[k // 8][:, lb + hf * P:lb + (hf + 1) * P],
                                rhs=rhs,
                                start=(k == 0), stop=(k == KC - 1))
                            nc.tensor.matmul(
                                out=p_v[:],
                                lhsT=wu_h[k // 8][:, lb + (2 + hf) * P:
                                                  lb + (3 + hf) * P],
                                rhs=rhs,
                                start=(k == 0), stop=(k == KC - 1))
                        sg = sg_pool.tile([P, w_], F32, tag="sg",
                                          name=f"sg_{j}_{sb}_{hf}")
                        nc.scalar.activation(sg[:], p_g[:],
                                             mybir.ActivationFunctionType.Silu)
                        nc.vector.tensor_mul(
                            h[:, hf * w_:(hf + 1) * w_], sg[:], p_v[:])
                    # down: out = [128 of C, w_ slots] per C chunk cc
                    for cc in range(KC):
                        pd = pd_pool.tile([P, w_], F32, space="PSUM", tag="pd",
                                          name=f"pd_{j}_{sb}_{cc}")
                        for kh in range(2):
                            nc.tensor.matmul(
                                out=pd[:],
                                lhsT=wd[:, kh * C + cc * P:kh * C + (cc + 1) * P],
                                rhs=h[:, kh * w_:(kh + 1) * w_],
                                start=(kh == 0), stop=(kh == 1))
                        dst = yo[:, cc * cap + sb:cc * cap + sb + w_]
                        if cc % 2 == 0:
                            nc.vector.tensor_copy(out=dst, in_=pd[:])
                        else:
                            nc.scalar.copy(dst, pd[:])
                        if sb + w_ == cap and cc in (7, KC - 1):
                            lo = 0 if cc == 7 else 8 * cap
                            nc.scalar.dma_start(
                                out=yrh[:, base + lo:base + lo + 8 * cap],
                                in_=yo[:, lo:lo + 8 * cap])
    return nc


# --------------------------------------------------------------------------
# Host wrapper
# --------------------------------------------------------------------------
def _pm(a, nchunk):
    """[nchunk*128, X] row-major -> partition-major [128, nchunk*X]."""
    x = a.shape[1]
    return np.ascontiguousarray(
        a.reshape(nchunk, P, x).transpose(1, 0, 2)).reshape(P, nchunk * x)


def kernel(x, w_gate, w_shared_up, w_shared_down, w_up, w_down):
    x_flat = x.reshape(-1, C)

    # ---- gate: sigmoid scores, top-6, normalized weights (f64 for a stable
    # ordering; ties in the fp32 reference are measure-zero) ----
    logits = x_flat.astype(np.float64) @ w_gate.astype(np.float64)
    scores = 1.0 / (1.0 + np.exp(-logits))
    topk_idx = np.argsort(-scores, axis=-1, kind="stable")[:, :TOPK]
    w = np.take_along_axis(scores, topk_idx, axis=-1)
    w = w / w.sum(-1, keepdims=True)

    # ---- dispatch positions (stable within each expert, slot-major order) --
    flat_e = topk_idx.reshape(-1)
    order = np.argsort(flat_e, kind="stable")
    sorted_e = flat_e[order]
    group_start = np.searchsorted(sorted_e, np.arange(E))
    counts = np.bincount(flat_e, minlength=E)

    token_of_slot = np.arange(N * TOPK) // TOPK
    expert_slots = []   # flat (token,k) slot ids, dispatch order, per expert
    expert_tokens = []
    for e in range(E):
        slots = order[group_start[e]: group_start[e] + counts[e]]
        expert_slots.append(slots)
        expert_tokens.append(token_of_slot[slots])

    # ---- balanced expert->core assignment: sort by count desc, deal 8 at a
    # time; slot position j has the same padded cap on every core ----
    ranks = np.argsort(-counts, kind="stable")
    expert_of = [[int(ranks[8 * j + c]) for j in range(ELOC)]
                 for c in range(NCORES)]
    caps = tuple(
        max(8, int(-(-int(counts[ranks[8 * j]]) // 8) * 8)) for j in range(ELOC))
    offs = [0]
    for cp in caps:
        offs.append(offs[-1] + cp)

    # ---- build per-core inputs (partition-major bf16) ----
    xT_bf = np.ascontiguousarray(x_flat.T).astype(_BF16_NP)  # [C, N]
    wsu_f = w_shared_up.astype(_BF16_NP)
    wsd_f = w_shared_down.astype(_BF16_NP)

    # shared-up weights in pair/k-major order (see _MPAIR)
    wr = wsu_f.reshape(KC, P, 8, P)[:, :, _MPAIR, :]        # [k, p, 8, 128]
    wsuh = np.ascontiguousarray(
        wr.reshape(KC, P, 4, 2 * P).transpose(1, 2, 0, 3)).reshape(P, KC * 2 * HS)
    wsdh = _pm(wsd_f, 4)

    in_maps = []
    for c in range(NCORES):
        xd_blocks = []
        wu_blocks = []
        wd_blocks = []
        for j in range(ELOC):
            e = expert_of[c][j]
            tok = expert_tokens[e]
            n = len(tok)
            blk = np.zeros((P, KC, caps[j]), dtype=_BF16_NP)
            blk[:, :, :n] = xT_bf[:, tok].reshape(KC, P, n).transpose(1, 0, 2)
            xd_blocks.append(blk.reshape(P, -1))
            wu_blocks.append(_pm(w_up[e].astype(_BF16_NP), KC))
            wd_blocks.append(_pm(w_down[e].astype(_BF16_NP), 2))
        xsh = _pm(np.ascontiguousarray(
            xT_bf[:, c * NLOC:(c + 1) * NLOC]), KC)
        in_maps.append({
            "xdh": np.concatenate(xd_blocks, axis=1),
            "wuh": np.concatenate(wu_blocks, axis=1),
            "wdh": np.concatenate(wd_blocks, axis=1),
            "xsh": xsh,
            "wsuh": wsuh,
            "wsdh": wsdh,
        })

    if caps not in _BUILD_CACHE:
        _BUILD_CACHE[caps] = _build(caps)
    nc = _BUILD_CACHE[caps]

    res = run_bass_kernel_spmd(nc, in_maps, list(range(NCORES)))
    if res.exec_time_ns is not None:
        print(f"HW exec time: {res.exec_time_ns} ns", flush=True)

    # ---- host combine (unshard): gather per-slot rows, weight, sum ----
    y_ts = np.empty((N * TOPK, C), dtype=np.float32)
    for c in range(NCORES):
        yr = res.results[c]["yrh"]
        for j in range(ELOC):
            e = expert_of[c][j]
            n = int(counts[e])
            seg = yr[:, KC * offs[j]:KC * offs[j] + KC * caps[j]]
            seg = seg.reshape(P, KC, caps[j])[:, :, :n]
            y_ts[expert_slots[e]] = (
                seg.transpose(2, 1, 0).reshape(n, C).astype(np.float32))
    routed = (y_ts.reshape(N, TOPK, C)
              * w.reshape(N, TOPK, 1).astype(np.float32)).sum(axis=1)
    shared = np.concatenate(
        [r["ysh"] for r in res.results], axis=0).astype(np.float32)
    return (shared + routed).reshape(B, T, C).astype(np.float32)
